# revision 1
# baseline (speedup 1.0000x reference)
"""Distributed 4-layer GCN forward on 8 Trainium2 NeuronCores (Bass/Tile).

Strategy:
- Nodes are packed into 8 cores x 104 windows of 128 dst slots; per layer each
  core aggregates its windows: dma_gather pulls source rows (dis[src]*X[src],
  bf16, 256B rows) from a replicated gather table in HBM; a per-window segment
  matrix (one-hot with value dis[dst]) reduces edges to dst rows via TensorE
  matmuls accumulating in PSUM; the layer's dense matmul + bias + relu +
  dis-scale produce the core's output shard; a 2-way split AllGather rebuilds
  the next layer's gather table on every core.
- Self-loops are handled densely: one extra matmul per window with the
  window's own rows (sequential load, no gather descriptors) against a
  diagonal segment-matrix group.
- Q7 descriptor generation (~7.5ns/idx) is the bottleneck, so the node->window
  packing balances, per window, the edge counts from each of the 4 gather
  base blocks (int16 index limit) under a 512-edge cap to minimize padding;
  block membership is keyed on (core//4, half) so it is stable under packing.
"""
import sys

sys.path.insert(0, "/opt/trn_rl_repo")

import numpy as np
import ml_dtypes

NCORES = 8
N = 100000
WPC = 104            # windows per core
HALF_W = 52          # windows per AllGather half
NPC = WPC * 128      # 13312
NPAD = NCORES * NPC  # 106496
BLKSZ = 26624
BLK_BOUNDS = [0, 26624, 53248, 79872, 106496]
NBLK = 4
CHUNKS = [4096, 4096, 4096, 4096]
CELL_CAP = 512
DIMS = [64, 96, 64, 32, 16]
BF16 = ml_dtypes.bfloat16


# ----------------------------------------------------------------------------
# host preprocessing
# ----------------------------------------------------------------------------
def _pack_windows(vecs, order):
    """Greedy capacity-capped packing of nodes (rows of vecs[:,4]) into
    HALF_W windows of <=128 slots, balancing per-block sums under CELL_CAP."""
    sums = np.zeros((HALF_W, NBLK), np.int64)
    cnt = np.zeros(HALF_W, np.int64)
    win = np.empty(len(order), np.int64)
    for i in order:
        v = vecs[i]
        over = np.maximum(sums + v[None, :] - CELL_CAP, 0).sum(axis=1)
        over[cnt >= 128] = 1 << 30
        score = over * (1 << 20) + sums[:, int(np.argmax(v))]
        w = int(np.argmin(score))
        win[i] = w
        sums[w] += v
        cnt[w] += 1
    return win, sums


def _preprocess(x, edge_index):
    src = np.asarray(edge_index[0], dtype=np.int64)
    dst = np.asarray(edge_index[1], dtype=np.int64)

    deg = np.bincount(dst, minlength=N).astype(np.float32) + 1.0
    dis = (1.0 / np.sqrt(deg)).astype(np.float32)

    # --- phase A: snake deal by degree -> (core, half). fixes src blocks ---
    NBINS = NCORES * WPC
    order = np.argsort(-deg, kind="stable")
    k = np.arange(N)
    r = k // NBINS
    pos = k % NBINS
    binid = np.where(r % 2 == 0, pos, NBINS - 1 - pos)
    node_c = np.empty(N, np.int64)
    node_h = np.empty(N, np.int64)
    node_c[order] = binid % NCORES
    node_h[order] = (binid // NCORES) // HALF_W
    node_blk = node_h * 2 + node_c // 4          # gather block of each node

    # --- per-node in-degree split by src block ---
    vecs = np.zeros((N, NBLK), np.int64)
    np.add.at(vecs, (dst, node_blk[src]), 1)

    # --- phase B/C: per (core, half) packing + heaviest-first window order ---
    node_w = np.empty(N, np.int64)
    node_s = np.empty(N, np.int64)
    for c in range(NCORES):
        for h in range(2):
            sel = np.nonzero((node_c == c) & (node_h == h))[0]
            sub = vecs[sel]
            ordr = np.argsort(-sub.sum(axis=1), kind="stable")
            win, sums = _pack_windows(sub, ordr)
            tot = sums.sum(axis=1)
            worder = np.argsort(-tot, kind="stable")
            wrank = np.empty(HALF_W, np.int64)
            wrank[worder] = np.arange(HALF_W)
            wloc = wrank[win]
            node_w[sel] = h * HALF_W + wloc
            for wdx in range(HALF_W):
                ww = np.nonzero(wloc == wdx)[0]
                node_s[sel[ww]] = np.arange(len(ww))
    node_l = node_w * 128 + node_s
    gp = node_h * (HALF_W * 128 * NCORES) + node_c * (HALF_W * 128) + \
        (node_w % HALF_W) * 128 + node_s

    # --- edges (no self loops; those are the dense diagonal group) ---
    e_c = node_c[dst]
    e_w = node_w[dst]
    e_col = node_s[dst]
    e_gp_src = gp[src]
    e_b = e_gp_src // BLKSZ
    e_lidx = e_gp_src % BLKSZ
    e_val = dis[dst]

    key = (e_c * WPC + e_w) * NBLK + e_b
    counts = np.bincount(key, minlength=NCORES * WPC * NBLK).reshape(NCORES, WPC, NBLK)
    G = np.ceil(counts.max(axis=0) / 128.0).astype(np.int64)      # [WPC, NBLK]
    Gtot = G.sum(axis=1)
    gs_base = np.zeros((NBLK, WPC), np.int64)
    for b in range(NBLK):
        gs_base[b] = np.concatenate([[0], np.cumsum(G[:, b])[:-1]]) * 128
    S = G.sum(axis=0) * 128
    Spad = [int(-(-S[b] // CHUNKS[b]) * CHUNKS[b]) for b in range(NBLK)]
    # one-hot consumption layout: per window (Gtot_w + 1) groups, diag last
    GD = Gtot + 1
    cons_base_w = np.concatenate([[0], np.cumsum(GD)[:-1]]) * 128
    cons_base = cons_base_w[:, None] + np.concatenate(
        [np.zeros((WPC, 1), np.int64), np.cumsum(G[:, :-1], axis=1)], axis=1) * 128
    TOT = int(GD.sum() * 128)
    oh_off = np.zeros(WPC + 1, np.int64)
    for w in range(WPC):
        oh_off[w + 1] = oh_off[w] + 128 * int(GD[w]) * 128

    sort_idx = np.argsort(key, kind="stable")
    sorted_key = key[sort_idx]
    run_start = np.searchsorted(sorted_key, np.arange(NCORES * WPC * NBLK))
    rank = np.empty(len(key), np.int64)
    rank[sort_idx] = np.arange(len(key)) - run_start[sorted_key]

    idx_wrapped = []
    oh_dev = []
    dis_own_dev = []
    for c in range(NCORES):
        m = e_c == c
        w_, b_, col_, lidx_, val_, rk_ = e_w[m], e_b[m], e_col[m], e_lidx[m], e_val[m], rank[m]
        streams = []
        for b in range(NBLK):
            arr = np.zeros(Spad[b], np.int16)
            mb = b_ == b
            arr[gs_base[b][w_[mb]] + rk_[mb]] = lidx_[mb].astype(np.int16)
            streams.append(np.tile(arr.reshape(-1, 16).T, (8, 1)).copy())
        idx_wrapped.append(streams)

        do = np.zeros(NPC, np.float32)
        mc = node_c == c
        do[node_l[mc]] = dis[mc]

        oh = np.zeros((TOT, 128), BF16)
        slot = cons_base[w_, b_] + rk_
        oh[slot, col_] = val_.astype(BF16)
        # diagonal self-loop group per window (last group)
        drows = (cons_base_w + Gtot * 128)[:, None] + np.arange(128)[None, :]
        oh[drows.ravel(), np.tile(np.arange(128), WPC)] = do.astype(BF16)
        flat = np.empty(oh_off[-1], BF16)
        for w in range(WPC):
            cb = cons_base_w[w]
            gd_w = int(GD[w])
            blk = oh[cb:cb + gd_w * 128].reshape(gd_w, 128, 128).transpose(1, 0, 2)
            flat[oh_off[w]:oh_off[w + 1]] = blk.reshape(-1)
        oh_dev.append(flat)
        dis_own_dev.append(do.reshape(WPC, 128).T.copy())

    g1 = np.zeros((NPAD, 128), BF16)
    g1[gp, :x.shape[1]] = (np.asarray(x) * dis[:, None]).astype(BF16)
    x_own = []
    for c in range(NCORES):
        xo = np.zeros((NPC, 128), BF16)
        mc = node_c == c
        xo[node_l[mc]] = g1[gp[mc]]
        x_own.append(xo)

    meta = dict(G=G, Gtot=Gtot, GD=GD, gs_base=gs_base, S=S, Spad=Spad, TOT=TOT,
                oh_off=oh_off, node_c=node_c, node_l=node_l)
    return meta, g1, x_own, idx_wrapped, oh_dev, dis_own_dev


# ----------------------------------------------------------------------------
# bass program
# ----------------------------------------------------------------------------
def _build_program(meta):
    import os
    import concourse.mybir as mybir
    import concourse.tile as tile
    from concourse import bacc

    NLAYERS = int(os.environ.get("GCN_LAYERS", "4"))
    USE_AG = os.environ.get("GCN_AG", "1") == "1"
    NWIN = int(os.environ.get("GCN_WINDOWS", str(WPC)))

    G = meta["G"]; Gtot = meta["Gtot"]; gs_base = meta["gs_base"]
    Spad = meta["Spad"]; oh_off = meta["oh_off"]
    GD_MAX = int(meta["GD"].max())

    nc = bacc.Bacc(None)
    dt = mybir.dt

    g1 = nc.declare_dram_parameter("g1", [NPAD, 128], dt.bfloat16, isOutput=False)
    xop = nc.declare_dram_parameter("x_own", [NPC, 128], dt.bfloat16, isOutput=False)
    idxp = [nc.declare_dram_parameter(f"idx{b}", [128, Spad[b] // 16], dt.int16, isOutput=False)
            for b in range(NBLK)]
    ohp = nc.declare_dram_parameter("oh", [int(oh_off[-1])], dt.bfloat16, isOutput=False)
    Wp = [nc.declare_dram_parameter(f"W{i}", [DIMS[i], DIMS[i + 1]], dt.bfloat16, isOutput=False)
          for i in range(4)]
    brp = [nc.declare_dram_parameter(f"br{i}", [128, DIMS[i + 1]], dt.float32, isOutput=False)
           for i in range(4)]
    disp = nc.declare_dram_parameter("disown", [128, WPC], dt.float32, isOutput=False)
    outp = nc.declare_dram_parameter("outp", [NPC, 16], dt.float32, isOutput=True)

    shard = [nc.dram_tensor(f"shard{l}", [NPC, 128], dt.bfloat16) for l in range(3)]
    gts = [nc.dram_tensor(f"gt{l}", [NPAD, 128], dt.bfloat16, addr_space="Shared")
           for l in range(3)]
    HROW = HALF_W * 128          # 6656 rows per half-shard
    HOUT = HROW * NCORES         # 53248 rows per gather-table half

    with tile.TileContext(nc) as tc:
        with (
            tc.tile_pool(name="const", bufs=1) as cpool,
            tc.tile_pool(name="oh", bufs=3) as ohpool,
            tc.tile_pool(name="gat", bufs=2) as gpool,
            tc.tile_pool(name="xw", bufs=3) as xwpool,
            tc.tile_pool(name="zt", bufs=4) as zpool,
            tc.tile_pool(name="eps", bufs=4) as epool,
            tc.tile_pool(name="pt", bufs=4, space="PSUM") as ptpool,
            tc.tile_pool(name="ot", bufs=4, space="PSUM") as otpool,
        ):
            idx_t = []
            for b in range(NBLK):
                t = cpool.tile([128, Spad[b] // 16], dt.int16, tag=f"idx{b}")
                nc.sync.dma_start(out=t[:], in_=idxp[b][:])
                idx_t.append(t)
            dis_t = cpool.tile([128, WPC], dt.float32, tag="disown")
            nc.sync.dma_start(out=dis_t[:], in_=disp[:])
            W_t, br_t = [], []
            for i in range(4):
                wt = cpool.tile([DIMS[i], DIMS[i + 1]], dt.bfloat16, tag=f"W{i}")
                nc.sync.dma_start(out=wt[:], in_=Wp[i][:])
                W_t.append(wt)
                bt = cpool.tile([128, DIMS[i + 1]], dt.float32, tag=f"br{i}")
                nc.sync.dma_start(out=bt[:], in_=brp[i][:])
                br_t.append(bt)

            for l in range(NLAYERS):
                FIN, FOUT = DIMS[l], DIMS[l + 1]
                gt_src = g1 if l == 0 else gts[l - 1]
                own_src = xop if l == 0 else shard[l - 1]
                mtiles = [dict() for _ in range(NBLK)]

                def msgs_for(b, gpos, gt_src=gt_src, mtiles=mtiles):
                    ch = gpos // CHUNKS[b]
                    if ch not in mtiles[b]:
                        t = gpool.tile([128, CHUNKS[b] // 128, 128], dt.bfloat16,
                                       tag=f"msgs{b}")
                        c0 = ch * CHUNKS[b] // 16
                        nc.gpsimd.dma_gather(
                            out_ap=t[:],
                            in_ap=gt_src[BLK_BOUNDS[b]:BLK_BOUNDS[b + 1], :],
                            idxs_ap=idx_t[b][:, c0:c0 + CHUNKS[b] // 16],
                            num_idxs=CHUNKS[b],
                            num_idxs_reg=CHUNKS[b],
                            elem_size=128,
                            single_packet=False,
                        )
                        mtiles[b][ch] = t
                    return mtiles[b][ch], (gpos % CHUNKS[b]) // 128

                for w in range(NWIN):
                    gt_w = int(Gtot[w])
                    oh_t = ohpool.tile([128, GD_MAX, 128], dt.bfloat16, tag="oh")
                    nc.sync.dma_start(
                        out=oh_t[:, :gt_w + 1, :],
                        in_=ohp[int(oh_off[w]):int(oh_off[w + 1])].rearrange(
                            "(p x) -> p x", p=128),
                    )
                    xw = xwpool.tile([128, 128], dt.bfloat16, tag="xw")
                    nc.sync.dma_start(out=xw[:], in_=own_src[w * 128:(w + 1) * 128, :])
                    pt = ptpool.tile([FIN, 128], dt.float32, tag="pt")
                    gi = 0
                    for b in range(NBLK):
                        for g in range(int(G[w][b])):
                            mt, off = msgs_for(b, int(gs_base[b][w]) + g * 128)
                            nc.tensor.matmul(
                                pt[:], mt[:, off, :FIN], oh_t[:, gi, :],
                                start=(gi == 0), stop=False)
                            gi += 1
                    nc.tensor.matmul(pt[:], xw[:, :FIN], oh_t[:, gt_w, :],
                                     start=(gi == 0), stop=True)
                    zt = zpool.tile([FIN, 128], dt.bfloat16, tag="zt")
                    nc.scalar.activation(zt[:], pt[:], mybir.ActivationFunctionType.Copy)
                    ot = otpool.tile([128, FOUT], dt.float32, tag="ot")
                    nc.tensor.matmul(ot[:], zt[:], W_t[l][:], start=True, stop=True)
                    if l < 3:
                        t1 = epool.tile([128, FOUT], dt.float32, tag="t1")
                        nc.vector.tensor_tensor(out=t1[:], in0=ot[:], in1=br_t[l][:],
                                                op=mybir.AluOpType.add)
                        res = epool.tile([128, 128], dt.bfloat16, tag="res")
                        nc.scalar.activation(res[:, :FOUT], t1[:],
                                             mybir.ActivationFunctionType.Relu,
                                             scale=dis_t[:, w:w + 1])
                        nc.sync.dma_start(out=shard[l][w * 128:(w + 1) * 128, :FOUT],
                                          in_=res[:, :FOUT])
                        if w == HALF_W - 1 and USE_AG and l < NLAYERS - 1:
                            nc.gpsimd.collective_compute(
                                "AllGather", mybir.AluOpType.bypass,
                                replica_groups=[list(range(NCORES))],
                                ins=[shard[l][0:HROW, :]],
                                outs=[gts[l][0:HOUT, :]],
                            )
                    else:
                        t1 = epool.tile([128, 16], dt.float32, tag="t1f")
                        nc.vector.tensor_tensor(out=t1[:], in0=ot[:], in1=br_t[l][:],
                                                op=mybir.AluOpType.add)
                        nc.sync.dma_start(out=outp[w * 128:(w + 1) * 128, :], in_=t1[:])
                if l < 3 and USE_AG and l < NLAYERS - 1:
                    nc.gpsimd.collective_compute(
                        "AllGather", mybir.AluOpType.bypass,
                        replica_groups=[list(range(NCORES))],
                        ins=[shard[l][HROW:, :]],
                        outs=[gts[l][HOUT:, :]],
                    )

    nc.finalize()
    return nc


# ----------------------------------------------------------------------------
# entry point
# ----------------------------------------------------------------------------
def kernel(x, edge_index, W1, b1, W2, b2, W3, b3, W4, b4, _debug=None):
    from concourse.bass_utils import run_bass_kernel_spmd

    x = np.asarray(x)
    meta, g1, x_own, idx_wrapped, oh_dev, dis_own_dev = _preprocess(x, edge_index)
    nc = _build_program(meta)

    Ws = [np.asarray(w).astype(BF16) for w in (W1, W2, W3, W4)]
    bs = [np.asarray(b).astype(np.float32) for b in (b1, b2, b3, b4)]
    in_maps = []
    for c in range(NCORES):
        m = {"g1": g1, "x_own": x_own[c], "oh": oh_dev[c], "disown": dis_own_dev[c]}
        for b in range(NBLK):
            m[f"idx{b}"] = idx_wrapped[c][b]
        for i in range(4):
            m[f"W{i}"] = Ws[i]
            m[f"br{i}"] = np.tile(bs[i][None, :], (128, 1))
        in_maps.append(m)

    kwargs = dict(_debug) if _debug else {}
    kwargs.pop("res", None)
    res = run_bass_kernel_spmd(nc, in_maps, list(range(NCORES)), **kwargs)

    full = np.zeros((N, 16), np.float32)
    for c in range(NCORES):
        mc = meta["node_c"] == c
        full[mc] = res.results[c]["outp"][meta["node_l"][mc]]
    if _debug is not None:
        _debug["res"] = res
    return full



# revision 3
# speedup vs baseline: 2.5462x; 2.5462x over previous
"""Distributed 4-layer GCN forward on 8 Trainium2 NeuronCores (Bass/Tile).

Strategy:
- Nodes are packed into 8 cores x 104 windows of 128 dst slots; per layer each
  core aggregates its windows: dma_gather pulls source rows (dis[src]*X[src],
  bf16, 256B rows) from a replicated gather table in HBM; a per-window segment
  matrix (one-hot with value dis[dst]) reduces edges to dst rows via TensorE
  matmuls accumulating in PSUM; the layer's dense matmul + bias + relu +
  dis-scale produce the core's output shard; a 2-way split AllGather rebuilds
  the next layer's gather table on every core.
- Self-loops are handled densely: one extra matmul per window with the
  window's own rows (sequential load, no gather descriptors) against a
  diagonal segment-matrix group.
- Q7 descriptor generation (~7.5ns/idx) is the bottleneck, so the node->window
  packing balances, per window, the edge counts from each of the 4 gather
  base blocks (int16 index limit) under a 512-edge cap to minimize padding;
  block membership is keyed on (core//4, half) so it is stable under packing.
"""
import sys

sys.path.insert(0, "/opt/trn_rl_repo")

import numpy as np
import ml_dtypes

NCORES = 8
N = 100000
WPC = 104            # windows per core
HALF_W = 52          # windows per AllGather half
NPC = WPC * 128      # 13312
NPAD = NCORES * NPC  # 106496
BLKSZ = 26624
BLK_BOUNDS = [0, 26624, 53248, 79872, 106496]
NBLK = 4
CHUNKS = [4096, 4096, 4096, 4096]
CELL_CAP = 512
DIMS = [64, 96, 64, 32, 16]
BF16 = ml_dtypes.bfloat16


# ----------------------------------------------------------------------------
# host preprocessing
# ----------------------------------------------------------------------------
def _pack_windows(vecs, order):
    """Greedy capacity-capped packing of nodes (rows of vecs[:,4]) into
    HALF_W windows of <=128 slots, balancing per-block sums under CELL_CAP."""
    sums = np.zeros((HALF_W, NBLK), np.int64)
    cnt = np.zeros(HALF_W, np.int64)
    win = np.empty(len(order), np.int64)
    for i in order:
        v = vecs[i]
        over = np.maximum(sums + v[None, :] - CELL_CAP, 0).sum(axis=1)
        over[cnt >= 128] = 1 << 30
        score = over * (1 << 20) + sums[:, int(np.argmax(v))]
        w = int(np.argmin(score))
        win[i] = w
        sums[w] += v
        cnt[w] += 1
    return win, sums


def _preprocess(x, edge_index):
    src = np.asarray(edge_index[0], dtype=np.int64)
    dst = np.asarray(edge_index[1], dtype=np.int64)

    deg = np.bincount(dst, minlength=N).astype(np.float32) + 1.0
    dis = (1.0 / np.sqrt(deg)).astype(np.float32)

    # --- phase A: snake deal by degree -> (core, half). fixes src blocks ---
    NBINS = NCORES * WPC
    order = np.argsort(-deg, kind="stable")
    k = np.arange(N)
    r = k // NBINS
    pos = k % NBINS
    binid = np.where(r % 2 == 0, pos, NBINS - 1 - pos)
    node_c = np.empty(N, np.int64)
    node_h = np.empty(N, np.int64)
    node_c[order] = binid % NCORES
    node_h[order] = (binid // NCORES) // HALF_W
    node_blk = node_h * 2 + node_c // 4          # gather block of each node

    # --- per-node in-degree split by src block ---
    vecs = np.zeros((N, NBLK), np.int64)
    np.add.at(vecs, (dst, node_blk[src]), 1)

    # --- phase B/C: per (core, half) packing + heaviest-first window order ---
    node_w = np.empty(N, np.int64)
    node_s = np.empty(N, np.int64)
    for c in range(NCORES):
        for h in range(2):
            sel = np.nonzero((node_c == c) & (node_h == h))[0]
            sub = vecs[sel]
            ordr = np.argsort(-sub.sum(axis=1), kind="stable")
            win, sums = _pack_windows(sub, ordr)
            tot = sums.sum(axis=1)
            worder = np.argsort(-tot, kind="stable")
            wrank = np.empty(HALF_W, np.int64)
            wrank[worder] = np.arange(HALF_W)
            wloc = wrank[win]
            node_w[sel] = h * HALF_W + wloc
            for wdx in range(HALF_W):
                ww = np.nonzero(wloc == wdx)[0]
                node_s[sel[ww]] = np.arange(len(ww))
    node_l = node_w * 128 + node_s
    gp = node_h * (HALF_W * 128 * NCORES) + node_c * (HALF_W * 128) + \
        (node_w % HALF_W) * 128 + node_s

    # --- edges (no self loops; those are the dense diagonal group) ---
    e_c = node_c[dst]
    e_w = node_w[dst]
    e_col = node_s[dst]
    e_gp_src = gp[src]
    e_b = e_gp_src // BLKSZ
    e_lidx = e_gp_src % BLKSZ
    e_val = dis[dst]

    key = (e_c * WPC + e_w) * NBLK + e_b
    counts = np.bincount(key, minlength=NCORES * WPC * NBLK).reshape(NCORES, WPC, NBLK)
    G = np.ceil(counts.max(axis=0) / 128.0).astype(np.int64)      # [WPC, NBLK]
    Gtot = G.sum(axis=1)
    gs_base = np.zeros((NBLK, WPC), np.int64)
    for b in range(NBLK):
        gs_base[b] = np.concatenate([[0], np.cumsum(G[:, b])[:-1]]) * 128
    S = G.sum(axis=0) * 128
    Spad = [int(-(-S[b] // CHUNKS[b]) * CHUNKS[b]) for b in range(NBLK)]
    # one-hot consumption layout: per window (Gtot_w + 1) groups, diag last
    GD = Gtot + 1
    cons_base_w = np.concatenate([[0], np.cumsum(GD)[:-1]]) * 128
    cons_base = cons_base_w[:, None] + np.concatenate(
        [np.zeros((WPC, 1), np.int64), np.cumsum(G[:, :-1], axis=1)], axis=1) * 128
    TOT = int(GD.sum() * 128)
    oh_off = np.zeros(WPC + 1, np.int64)
    for w in range(WPC):
        oh_off[w + 1] = oh_off[w] + 128 * int(GD[w]) * 128

    sort_idx = np.argsort(key, kind="stable")
    sorted_key = key[sort_idx]
    run_start = np.searchsorted(sorted_key, np.arange(NCORES * WPC * NBLK))
    rank = np.empty(len(key), np.int64)
    rank[sort_idx] = np.arange(len(key)) - run_start[sorted_key]

    idx_wrapped = []
    oh_dev = []
    dis_own_dev = []
    for c in range(NCORES):
        m = e_c == c
        w_, b_, col_, lidx_, val_, rk_ = e_w[m], e_b[m], e_col[m], e_lidx[m], e_val[m], rank[m]
        streams = []
        for b in range(NBLK):
            arr = np.zeros(Spad[b], np.int16)
            mb = b_ == b
            arr[gs_base[b][w_[mb]] + rk_[mb]] = lidx_[mb].astype(np.int16)
            streams.append(np.tile(arr.reshape(-1, 16).T, (8, 1)).copy())
        idx_wrapped.append(streams)

        do = np.zeros(NPC, np.float32)
        mc = node_c == c
        do[node_l[mc]] = dis[mc]

        oh = np.zeros((TOT, 128), BF16)
        slot = cons_base[w_, b_] + rk_
        oh[slot, col_] = val_.astype(BF16)
        # diagonal self-loop group per window (last group)
        drows = (cons_base_w + Gtot * 128)[:, None] + np.arange(128)[None, :]
        oh[drows.ravel(), np.tile(np.arange(128), WPC)] = do.astype(BF16)
        flat = np.empty(oh_off[-1], BF16)
        for w in range(WPC):
            cb = cons_base_w[w]
            gd_w = int(GD[w])
            blk = oh[cb:cb + gd_w * 128].reshape(gd_w, 128, 128).transpose(1, 0, 2)
            flat[oh_off[w]:oh_off[w + 1]] = blk.reshape(-1)
        oh_dev.append(flat)
        dis_own_dev.append(do.reshape(WPC, 128).T.copy())

    g1 = np.zeros((NPAD, 128), BF16)
    g1[gp, :x.shape[1]] = (np.asarray(x) * dis[:, None]).astype(BF16)
    x_own = []
    for c in range(NCORES):
        xo = np.zeros((NPC, 128), BF16)
        mc = node_c == c
        xo[node_l[mc]] = g1[gp[mc]]
        x_own.append(xo)

    meta = dict(G=G, Gtot=Gtot, GD=GD, gs_base=gs_base, S=S, Spad=Spad, TOT=TOT,
                oh_off=oh_off, node_c=node_c, node_l=node_l)
    return meta, g1, x_own, idx_wrapped, oh_dev, dis_own_dev


# ----------------------------------------------------------------------------
# bass program
# ----------------------------------------------------------------------------
def _build_program(meta):
    import os
    import concourse.mybir as mybir
    import concourse.tile as tile
    from concourse import bacc

    NLAYERS = int(os.environ.get("GCN_LAYERS", "4"))
    USE_AG = os.environ.get("GCN_AG", "1") == "1"
    NWIN = int(os.environ.get("GCN_WINDOWS", str(WPC)))

    G = meta["G"]; Gtot = meta["Gtot"]; gs_base = meta["gs_base"]
    Spad = meta["Spad"]; oh_off = meta["oh_off"]
    GD_MAX = int(meta["GD"].max())

    nc = bacc.Bacc(None, num_swdge_queues=4)
    dt = mybir.dt

    g1 = nc.declare_dram_parameter("g1", [NPAD, 128], dt.bfloat16, isOutput=False)
    xop = nc.declare_dram_parameter("x_own", [NPC, 128], dt.bfloat16, isOutput=False)
    idxp = [nc.declare_dram_parameter(f"idx{b}", [128, Spad[b] // 16], dt.int16, isOutput=False)
            for b in range(NBLK)]
    ohp = nc.declare_dram_parameter("oh", [int(oh_off[-1])], dt.bfloat16, isOutput=False)
    Wp = [nc.declare_dram_parameter(f"W{i}", [DIMS[i], DIMS[i + 1]], dt.bfloat16, isOutput=False)
          for i in range(4)]
    brp = [nc.declare_dram_parameter(f"br{i}", [128, DIMS[i + 1]], dt.float32, isOutput=False)
           for i in range(4)]
    disp = nc.declare_dram_parameter("disown", [128, WPC], dt.float32, isOutput=False)
    outp = nc.declare_dram_parameter("outp", [NPC, 16], dt.float32, isOutput=True)

    shard = [nc.dram_tensor(f"shard{l}", [NPC, 128], dt.bfloat16) for l in range(3)]
    gts = [nc.dram_tensor(f"gt{l}", [NPAD, 128], dt.bfloat16, addr_space="Shared")
           for l in range(3)]
    HROW = HALF_W * 128          # 6656 rows per half-shard
    HOUT = HROW * NCORES         # 53248 rows per gather-table half

    with tile.TileContext(nc) as tc:
        with (
            tc.tile_pool(name="const", bufs=1) as cpool,
            tc.tile_pool(name="oh", bufs=3) as ohpool,
            tc.tile_pool(name="gat", bufs=2) as gpool,
            tc.tile_pool(name="xw", bufs=3) as xwpool,
            tc.tile_pool(name="zt", bufs=4) as zpool,
            tc.tile_pool(name="eps", bufs=4) as epool,
            tc.tile_pool(name="pt", bufs=4, space="PSUM") as ptpool,
            tc.tile_pool(name="ot", bufs=4, space="PSUM") as otpool,
        ):
            idx_t = []
            for b in range(NBLK):
                t = cpool.tile([128, Spad[b] // 16], dt.int16, tag=f"idx{b}")
                nc.sync.dma_start(out=t[:], in_=idxp[b][:])
                idx_t.append(t)
            dis_t = cpool.tile([128, WPC], dt.float32, tag="disown")
            nc.sync.dma_start(out=dis_t[:], in_=disp[:])
            W_t, br_t = [], []
            for i in range(4):
                wt = cpool.tile([DIMS[i], DIMS[i + 1]], dt.bfloat16, tag=f"W{i}")
                nc.sync.dma_start(out=wt[:], in_=Wp[i][:])
                W_t.append(wt)
                bt = cpool.tile([128, DIMS[i + 1]], dt.float32, tag=f"br{i}")
                nc.sync.dma_start(out=bt[:], in_=brp[i][:])
                br_t.append(bt)

            for l in range(NLAYERS):
                FIN, FOUT = DIMS[l], DIMS[l + 1]
                gt_src = g1 if l == 0 else gts[l - 1]
                own_src = xop if l == 0 else shard[l - 1]
                mtiles = [dict() for _ in range(NBLK)]

                def msgs_for(b, gpos, gt_src=gt_src, mtiles=mtiles):
                    ch = gpos // CHUNKS[b]
                    if ch not in mtiles[b]:
                        t = gpool.tile([128, CHUNKS[b] // 128, 128], dt.bfloat16,
                                       tag=f"msgs{b}")
                        c0 = ch * CHUNKS[b] // 16
                        nc.gpsimd.dma_gather(
                            out_ap=t[:],
                            in_ap=gt_src[BLK_BOUNDS[b]:BLK_BOUNDS[b + 1], :],
                            idxs_ap=idx_t[b][:, c0:c0 + CHUNKS[b] // 16],
                            num_idxs=CHUNKS[b],
                            num_idxs_reg=CHUNKS[b],
                            elem_size=128,
                            single_packet=False,
                            queue_num=b,
                        )
                        mtiles[b][ch] = t
                    return mtiles[b][ch], (gpos % CHUNKS[b]) // 128

                for w in range(NWIN):
                    gt_w = int(Gtot[w])
                    oh_t = ohpool.tile([128, GD_MAX, 128], dt.bfloat16, tag="oh")
                    nc.sync.dma_start(
                        out=oh_t[:, :gt_w + 1, :],
                        in_=ohp[int(oh_off[w]):int(oh_off[w + 1])].rearrange(
                            "(p x) -> p x", p=128),
                    )
                    xw = xwpool.tile([128, 128], dt.bfloat16, tag="xw")
                    nc.sync.dma_start(out=xw[:], in_=own_src[w * 128:(w + 1) * 128, :])
                    pt = ptpool.tile([FIN, 128], dt.float32, tag="pt")
                    gi = 0
                    for b in range(NBLK):
                        for g in range(int(G[w][b])):
                            mt, off = msgs_for(b, int(gs_base[b][w]) + g * 128)
                            nc.tensor.matmul(
                                pt[:], mt[:, off, :FIN], oh_t[:, gi, :],
                                start=(gi == 0), stop=False)
                            gi += 1
                    nc.tensor.matmul(pt[:], xw[:, :FIN], oh_t[:, gt_w, :],
                                     start=(gi == 0), stop=True)
                    zt = zpool.tile([FIN, 128], dt.bfloat16, tag="zt")
                    nc.scalar.activation(zt[:], pt[:], mybir.ActivationFunctionType.Copy)
                    ot = otpool.tile([128, FOUT], dt.float32, tag="ot")
                    nc.tensor.matmul(ot[:], zt[:], W_t[l][:], start=True, stop=True)
                    if l < 3:
                        t1 = epool.tile([128, FOUT], dt.float32, tag="t1")
                        nc.vector.tensor_tensor(out=t1[:], in0=ot[:], in1=br_t[l][:],
                                                op=mybir.AluOpType.add)
                        res = epool.tile([128, 128], dt.bfloat16, tag="res")
                        nc.scalar.activation(res[:, :FOUT], t1[:],
                                             mybir.ActivationFunctionType.Relu,
                                             scale=dis_t[:, w:w + 1])
                        nc.sync.dma_start(out=shard[l][w * 128:(w + 1) * 128, :FOUT],
                                          in_=res[:, :FOUT])
                        if w == HALF_W - 1 and USE_AG and l < NLAYERS - 1:
                            nc.gpsimd.collective_compute(
                                "AllGather", mybir.AluOpType.bypass,
                                replica_groups=[list(range(NCORES))],
                                ins=[shard[l][0:HROW, :]],
                                outs=[gts[l][0:HOUT, :]],
                            )
                    else:
                        t1 = epool.tile([128, 16], dt.float32, tag="t1f")
                        nc.vector.tensor_tensor(out=t1[:], in0=ot[:], in1=br_t[l][:],
                                                op=mybir.AluOpType.add)
                        nc.sync.dma_start(out=outp[w * 128:(w + 1) * 128, :], in_=t1[:])
                if l < 3 and USE_AG and l < NLAYERS - 1:
                    nc.gpsimd.collective_compute(
                        "AllGather", mybir.AluOpType.bypass,
                        replica_groups=[list(range(NCORES))],
                        ins=[shard[l][HROW:, :]],
                        outs=[gts[l][HOUT:, :]],
                    )

    nc.finalize()
    return nc


# ----------------------------------------------------------------------------
# entry point
# ----------------------------------------------------------------------------
def kernel(x, edge_index, W1, b1, W2, b2, W3, b3, W4, b4, _debug=None):
    from concourse.bass_utils import run_bass_kernel_spmd

    x = np.asarray(x)
    meta, g1, x_own, idx_wrapped, oh_dev, dis_own_dev = _preprocess(x, edge_index)
    nc = _build_program(meta)

    Ws = [np.asarray(w).astype(BF16) for w in (W1, W2, W3, W4)]
    bs = [np.asarray(b).astype(np.float32) for b in (b1, b2, b3, b4)]
    in_maps = []
    for c in range(NCORES):
        m = {"g1": g1, "x_own": x_own[c], "oh": oh_dev[c], "disown": dis_own_dev[c]}
        for b in range(NBLK):
            m[f"idx{b}"] = idx_wrapped[c][b]
        for i in range(4):
            m[f"W{i}"] = Ws[i]
            m[f"br{i}"] = np.tile(bs[i][None, :], (128, 1))
        in_maps.append(m)

    kwargs = dict(_debug) if _debug else {}
    kwargs.pop("res", None)
    res = run_bass_kernel_spmd(nc, in_maps, list(range(NCORES)), **kwargs)

    full = np.zeros((N, 16), np.float32)
    for c in range(NCORES):
        mc = meta["node_c"] == c
        full[mc] = res.results[c]["outp"][meta["node_l"][mc]]
    if _debug is not None:
        _debug["res"] = res
    return full

